# revision 34
# baseline (speedup 1.0000x reference)
"""SAGAN-style attention block (nn_AttentionBlock) on 8 Trainium2 NeuronCores.

Problem (per sample): x [C=64, N=4096] (N = 64x64 spatial),
  f = Wf x + bf   [8, N]       g = Wg x + bg   [8, N]
  h = Wh x + bh   [64, N]
  s = f^T g       [N, N];  beta = softmax(s, axis=1)   (over j)
  o[c, i] = sum_j h[c, j] beta[i, j];   out = x + o
Sharding: pure data parallel over batch B=8 -> one sample per core.

Engine plan (all per core):
  - ACT is the natural bottleneck (16.7M exps at 153.6 G/s = 109us floor).
    ~25% of the exp work is offloaded to the DVE via two chained custom
    DVE ops registered at import time:
      EXP_BASE4_ANT: b = ((c0*s + c1)*s + c2)^4   (quadratic base, 6 stages)
      EXP_SQ8_ANT:   e = b^256                     (8 squarings)
    Together e = base^1024 ~ exp(s-12); rel err <1e-3 on significant
    entries (validated end-to-end: full-approx output rel err 2.5e-4).
  - All projections read x/weights as f32r via AP bitcast (no cast pass
    at all).  Input arrives as 8 x 512-col DMA pieces; f/g projections
    ([Wf;Wg] -> one [16,512] matmul per piece), hT tiles, and the main
    loop's first groups pipeline behind the DMA: first EXP at ~6us
    instead of ~34us.
  - A continuous dummy-matmul warmup stream from t~1us keeps the PE busy
    so the HAM clock gate (needs ~3.4us of sustained activity) opens
    before the real work, instead of 44us in.
  - PSUM (8 banks): psA 3 (s-macros + staged fg/hTa projections,
    single-buffered for chunk 0) + psB 3 (hTb staging j8-31, then joins
    s double-buffering from group 9) + po1 1 + po2 1 (o accumulators).
  - Finalize: merge accumulators + reciprocal on DVE; 1/den broadcast
    via DRAM bounce for chunks 0-6 (latency hidden), via a K=1 PE
    matmul into PSUM for the last chunk (kills the ~6.5us exposed tail);
    divide+residual on GpSimd (idle otherwise) for chunks 0-6.
  - o-matmuls of DVE-routed groups are deferred 3 groups (vs 1) in the
    emission pipeline so the PE never stalls on the slower DVE e-tiles.
    Accumulation order across j is irrelevant; j0/j1 (start) and j30/j31
    (stop) stay in the always-ACT groups 0 and 10.
"""
import sys

sys.path.insert(0, "/opt/trn_rl_repo")

import numpy as np
from contextlib import ExitStack

try:  # tracing hook is optional; provide a stub if absent
    import antenv.axon_hooks  # noqa: F401
except ImportError:
    import types
    import antenv
    _stub = types.ModuleType("antenv.axon_hooks")
    _stub._hook = None
    _stub.get_axon_ntff_profile_hook = lambda: _stub._hook
    def _set_hook(hook):
        _stub._hook = hook
    _stub.set_axon_ntff_profile_hook = _set_hook
    sys.modules["antenv.axon_hooks"] = _stub
    antenv.axon_hooks = _stub
    try:  # register the real ctypes NTFF hook (timing); degrade silently
        from trn_agent_boot.trn_boot import _ntff_profile_via_ctypes
        _stub._hook = _ntff_profile_via_ctypes("/opt/axon/libaxon_pjrt.so")
    except Exception:
        pass

import concourse.bass as bass  # noqa: F401  (bacc subclasses Bass)
import concourse.tile as tile
from concourse import bacc, mybir
from concourse.bass_utils import run_bass_kernel_spmd

# ---- custom DVE exp ops (registered at import) ----------------------------
from concourse.dve_ops import DveOp, OPS, CUSTOM_DVE_SPECS, _SUB_OPCODE_FOR_NAME
from concourse.dve_spec import Spec, Src0, C0, C1, C2, sq
from concourse.dve_spec import lower as _dve_lower
from concourse.dve_uop import DveOpSpec

F32 = mybir.dt.float32
F32R = mybir.dt.float32r
BF16 = mybir.dt.bfloat16
MMDT = BF16

EXPN = 1024.0  # exp(x) = base(x/EXPN)^EXPN, base = 1 + t + t^2/2
# base as a quadratic in s (x = s - 12 folded into the coefficients)
EB_C0 = float(0.5 / (EXPN * EXPN))
EB_C1 = float(1.0 / EXPN - 12.0 / (EXPN * EXPN))
EB_C2 = float(1.0 - 12.0 / EXPN + 72.0 / (EXPN * EXPN))


def _ref_exp_base4(in0, in1, s0, s1, imm2):
    x = in0.astype(np.float32)
    b = (np.float32(s0) * x + np.float32(s1)) * x + np.float32(imm2)
    return (b * b) * (b * b)


def _ref_sq8(in0, in1, s0, s1, imm2):
    b = in0.astype(np.float32)
    for _ in range(8):
        b = b * b
    return b


def _register_op(name, body, reference):
    if name in _SUB_OPCODE_FOR_NAME:  # idempotent across re-imports
        return next(op for op in OPS if op.name == name)
    spec = Spec(body=body, reference=reference)
    op = DveOp(name, spec, subdim=False, uops_sha={})
    object.__setattr__(op, "uops_sha", {
        ver: DveOpSpec(name=name, opcode=1, uops=_dve_lower(spec, ver=ver),
                       rd1_en=False).sha(ver)
        for ver in ("v3", "v4")})
    OPS.append(op)
    CUSTOM_DVE_SPECS[name] = spec
    _SUB_OPCODE_FOR_NAME[name] = max(_SUB_OPCODE_FOR_NAME.values()) + 1
    return op


EXP_BASE4 = _register_op(
    "EXP_BASE4_ANT", sq(sq((Src0 * C0 + C1) * Src0 + C2)), _ref_exp_base4)
EXP_SQ8 = _register_op(
    "EXP_SQ8_ANT", sq(sq(sq(sq(sq(sq(sq(sq(Src0)))))))), _ref_sq8)

B, C, H, W = 8, 64, 64, 64
N = H * W          # 4096
C8 = 8
NCORES = 8
ICHUNK = 512
NI = N // ICHUNK   # 8
JT = 128
NJ = N // JT       # 32
GROUP = 3
NPIECE = 8         # input x DMA pieces (512 cols each)
WN = N + 16 + C    # marshalled input width (x | wfg | wh)
NWARM = 7          # [128,512] dummy matmuls to open the HAM clock gate
# groups routed to the DVE exp (never 0 or 10: start/stop flags live there).
# chunks 0-1 stay on ACT: the DVE is busy with projection evac/replication
# during the prologue window.
DVE_GROUPS = {q: ((2, 5, 8) if q >= 3 else (3, 8)) for q in range(NI)}
DVE_GROUPS[0] = ()
O_DELAY_ACT = 1    # software-pipeline depth for o-matmul emission
O_DELAY_DVE = 3

_CACHE = {}


def _build_nc():
    nc = bacc.Bacc("TRN2", target_bir_lowering=False, debug=False,
                   num_devices=NCORES)
    inp = nc.dram_tensor("inp", [C + 1, WN], F32, kind="ExternalInput").ap()
    out = nc.dram_tensor("out", [C, N], F32, kind="ExternalOutput").ap()

    groups = []
    j0 = 0
    while j0 < NJ:
        groups.append((j0, min(GROUP, NJ - j0)))
        j0 += GROUP
    NG = len(groups)  # 11

    with tile.TileContext(nc) as tc:
        with ExitStack() as ctx:
            sb = ctx.enter_context(tc.tile_pool(name="sb", bufs=1))
            epool = ctx.enter_context(tc.tile_pool(name="ep", bufs=10))
            midp = ctx.enter_context(tc.tile_pool(name="mid", bufs=3))
            fin = ctx.enter_context(tc.tile_pool(name="fin", bufs=8))
            psA = ctx.enter_context(tc.tile_pool(name="psA", bufs=1, space="PSUM"))
            psB = ctx.enter_context(tc.tile_pool(name="psB", bufs=1, space="PSUM"))
            psO1 = ctx.enter_context(tc.tile_pool(name="psO1", bufs=1, space="PSUM"))
            psO2 = ctx.enter_context(tc.tile_pool(name="psO2", bufs=1, space="PSUM"))
            dram = ctx.enter_context(tc.tile_pool(name="dram", bufs=8, space="DRAM"))

            # ---- constants ----
            ones_f = sb.tile([128, 1], F32)
            nc.vector.memset(ones_f[:], 1.0)
            expbias = sb.tile([128, 1], F32)
            nc.vector.memset(expbias[:], -12.0)
            ones_c = sb.tile([1, C], F32)
            nc.vector.tensor_copy(ones_c[:], ones_f[0:1, 0:1].to_broadcast((1, C)))
            warm_src = sb.tile([128, 512], MMDT)
            nc.vector.tensor_copy(warm_src[:],
                                  ones_f[:, 0:1].to_broadcast((128, 512)))

            # ---- input DMA: x piece 0 first (it gates the first s-trio),
            # then weights, then the remaining pieces ----
            tin = sb.tile([C + 1, WN], F32)
            nc.sync.dma_start(tin[:, 0:ICHUNK], inp[:, 0:ICHUNK])
            nc.sync.dma_start(tin[:, N:WN], inp[:, N:WN])
            for c in range(1, NPIECE):
                cs = slice(c * ICHUNK, (c + 1) * ICHUNK)
                nc.sync.dma_start(tin[:, cs], inp[:, cs])
            # bf16 copies for the projection matmuls (f32r would need a
            # rounding producer: the BIR verifier rejects plain bitcasts).
            # Measured output rel-L2 with all-bf16 projections is ~3e-3,
            # well inside the 2e-2 gate.
            # [Wf | pad | Wg] stationary: f lands at PSUM partitions 0-7,
            # g at 32-39 (PSUM reads must start at a 32-aligned partition)
            wfg_b = sb.tile([C + 1, 40], MMDT)
            nc.vector.memset(wfg_b[:], 0.0)
            nc.vector.tensor_copy(wfg_b[:, 0:C8], tin[:, N:N + C8])
            nc.vector.tensor_copy(wfg_b[:, 32:40], tin[:, N + C8:N + 16])
            wh_b = sb.tile([C + 1, C], MMDT)
            nc.vector.tensor_copy(wh_b[:], tin[:, N + 16:WN])
            xb = sb.tile([C + 1, N], MMDT)

            # ---- SBUF landing zones ----
            # f/g replicated to partition offsets 0/32/64 (quadrant packing
            # needs both s-matmul operands at base partition 0/32/64)
            f_sb = sb.tile([72, N], MMDT)
            g_sb = sb.tile([72, N], MMDT)
            hT = sb.tile([JT, NJ, C + 1], MMDT)
            res = sb.tile([C, N], F32)

            # ---- warmup: continuous PE activity to open the HAM gate ----
            warm_ps = psO2.tile([JT, 512], F32, tag="po2", name="warm")
            for _ in range(NWARM):
                nc.tensor.matmul(warm_ps[:], warm_src[:, 0:JT], warm_src[:],
                                 start=True, stop=True)

            def pe_fill(n):
                # dummy stationary loads: pure PE activity (no PSUM/output)
                # to keep the HAM activity monitor from closing the clock
                # gate during engine-wait windows
                for _ in range(n):
                    nc.tensor.ldweights(warm_src[:, 0:JT])

            # ---- staged projections (psA pool, serialized with s-macros) ----
            def fg_piece(c):
                cs = slice(c * ICHUNK, (c + 1) * ICHUNK)
                nc.vector.tensor_copy(xb[:, cs], tin[:, cs])
                pool, tg = (psO1, "po1") if c % 2 == 0 else (psO2, "po2")
                pp = pool.tile([40, ICHUNK], F32, tag=tg, name=f"fg{c}")
                nc.tensor.matmul(pp[:], wfg_b[:], xb[:, cs],
                                 start=True, stop=True)
                # evac (the fp32->bf16 cast point); split ACT/DVE for the
                # first two pieces, which gate the first s-trio
                if c < 2:
                    nc.scalar.copy(f_sb[0:C8, cs], pp[0:C8, :])
                else:
                    nc.vector.tensor_copy(f_sb[0:C8, cs], pp[0:C8, :])
                nc.vector.tensor_copy(g_sb[0:C8, cs], pp[32:40, :])
                for r in (32, 64):
                    nc.vector.tensor_copy(f_sb[r:r + C8, cs], f_sb[0:C8, cs])
                    nc.vector.tensor_copy(g_sb[r:r + C8, cs], g_sb[0:C8, cs])
                pe_fill(3)

            # hT staging: four [128, 8*64] quarter-tiles through psB (its
            # s-macro duty starts only at group 6), serialized by the pool
            hq = {}

            def hT_mm(t, half):
                if half == 0:
                    hq[t] = psB.tile([JT, 8 * C], F32, tag="m", name=f"hq{t}")
                for j in range(8 * t + 4 * half, 8 * t + 4 * half + 4):
                    nc.tensor.matmul(hq[t][:, (j - 8 * t) * C:(j - 8 * t + 1) * C],
                                     xb[:, j * JT:(j + 1) * JT],
                                     wh_b[:], start=True, stop=True)
                pe_fill(2)

            def hT_evac(t):
                nc.vector.tensor_copy(
                    hT[:, 8 * t:8 * t + 8, 0:C],
                    hq[t][:].rearrange("p (a b) -> p a b", a=8))

            fg_piece(0)
            fg_piece(1)
            fg_piece(2)
            hT_mm(0, 0)
            hT_mm(0, 1)
            hT_evac(0)
            nc.vector.tensor_copy(hT[:, :, C:C + 1],
                                  ones_f[:].to_broadcast((JT, NJ, 1)))

            # prologue tasks injected between chunk-0 group emissions,
            # keyed by global group index
            inject = {
                0: [lambda: fg_piece(3), lambda: hT_mm(1, 0)],
                1: [lambda: fg_piece(4), lambda: hT_mm(1, 1),
                    lambda: hT_evac(1)],
                2: [lambda: fg_piece(5), lambda: hT_mm(2, 0)],
                3: [lambda: fg_piece(6), lambda: hT_mm(2, 1),
                    lambda: hT_evac(2)],
                4: [lambda: fg_piece(7), lambda: hT_mm(3, 0)],
                5: [lambda: hT_mm(3, 1), lambda: hT_evac(3)],
            }

            # ---- finalize helpers ----
            def fin_front(po, q):
                # merge the two o-accumulator halves (an engine may read
                # only ONE non-scalar PSUM input per instruction)
                oc = fin.tile([C + 1, ICHUNK], F32, tag="oc", name=f"oc{q}")
                nc.vector.tensor_copy(oc[:], po[0][:])
                nc.vector.tensor_add(oc[:], oc[:], po[1][:])
                # reciprocal of the WHOLE oc tile (base partition 0 keeps
                # the custom op well-behaved); only row C (the denominator)
                # is ever read -- rows 0-63 are 1/o garbage, never used
                ri = fin.tile([C + 1, ICHUNK], F32, tag="ri", name=f"ri{q}")
                nc.vector.reciprocal_approx_fast(ri[:], oc[:])
                r = ri[C:C + 1, :]
                if q < NI - 1:
                    scr = dram.tile([1, ICHUNK], F32, tag="scr", name=f"scr{q}")
                    nc.sync.dma_start(scr[:], r)
                    rb = fin.tile([C, ICHUNK], F32, tag="rb", name=f"rb{q}")
                    nc.sync.dma_start(rb[:], scr[:].to_broadcast((C, ICHUNK)))
                else:
                    # tail chunk: broadcast 1/den via a K=1 fp32 PE matmul
                    # into PSUM (psA is free once the last s-macro is read;
                    # fp32 streams 4 cyc/col but the PE is idle here).
                    # matmul operands must share a base partition -> move
                    # the reciprocal row to partition 0 first.
                    r0 = fin.tile([1, ICHUNK], F32, tag="r0", name="r0")
                    nc.vector.tensor_copy(r0[:], r)
                    rb = psA.tile([C, ICHUNK], F32, tag="m", name="rbps")
                    nc.tensor.matmul(rb[:], ones_c[:], r0[:],
                                     start=True, stop=True)
                return (oc, rb, q)

            def fin_back(oc, rb, q):
                qs = slice(q * ICHUNK, (q + 1) * ICHUNK)
                eng = nc.gpsimd if q < NI - 1 else nc.vector
                eng.tensor_mul(res[:, qs], oc[0:C, :], rb[:])
                eng.tensor_add(res[:, qs], res[:, qs], tin[0:C, qs])
                nc.sync.dma_start(out[:, qs], res[:, qs])

            # ---- main loop with deferred-o scheduler ----
            # po tiles are allocated lazily at the FIRST o-matmul emission
            # of each chunk: a pool's buffer rotation follows tile-creation
            # order, so allocating them at the chunk-loop top would place
            # them BEFORE later-injected staging tiles in psO1/psO2 and
            # deadlock (staging would wait on the chunk's merge).
            po_of = {}
            done_o = {q: 0 for q in range(NI)}
            oqueue = []  # (due_gidx, emit_fn, q) in queue order
            state = {"pend_back": None}

            def get_po(q):
                if q not in po_of:
                    po_of[q] = [
                        psO1.tile([C + 1, ICHUNK], F32, tag="po1",
                                  name=f"po1_{q}"),
                        psO2.tile([C + 1, ICHUNK], F32, tag="po2",
                                  name=f"po2_{q}"),
                    ]
                return po_of[q]

            def make_emit_o(q, e, j0_, glen_):
                def emit():
                    po = get_po(q)
                    for k in range(glen_):
                        j = j0_ + k
                        nc.tensor.matmul(po[j % 2][:], hT[:, j, :],
                                         e[:, k * ICHUNK:(k + 1) * ICHUNK],
                                         start=(j < 2), stop=(j >= NJ - 2))
                return emit

            def flush_o(now):
                while oqueue and oqueue[0][0] <= now:
                    _, fn, oq = oqueue.pop(0)
                    fn()
                    done_o[oq] += 1
                    if done_o[oq] == NG:
                        front = fin_front(po_of[oq], oq)
                        if state["pend_back"] is not None:
                            fin_back(*state["pend_back"])
                        state["pend_back"] = front

            gidx = 0
            for q in range(NI):
                qs = slice(q * ICHUNK, (q + 1) * ICHUNK)
                dset = DVE_GROUPS.get(q, ())
                for gi, (gj0, glen) in enumerate(groups):
                    # chunk 0 groups 0-5: psA single-buffered (psB stages hT)
                    if gidx < 6:
                        pool = psA
                    else:
                        pool = psB if (gidx - 6) % 2 == 0 else psA
                    pm = pool.tile([JT, GROUP * ICHUNK], F32, tag="m")
                    for k in range(glen):
                        j = gj0 + k
                        nc.tensor.matmul(
                            pm[:, k * ICHUNK:(k + 1) * ICHUNK],
                            g_sb[32 * k:32 * k + C8, j * JT:(j + 1) * JT],
                            f_sb[32 * k:32 * k + C8, qs],
                            start=True, stop=True)
                    e = epool.tile([JT, GROUP * ICHUNK], MMDT, tag="e")
                    if gi in dset:
                        mid = midp.tile([JT, GROUP * ICHUNK], F32, tag="mid")
                        nc.vector._custom_dve(
                            EXP_BASE4, out=mid[:, 0:glen * ICHUNK],
                            in0=pm[:, 0:glen * ICHUNK],
                            s0=EB_C0, s1=EB_C1, imm2=EB_C2)
                        nc.vector._custom_dve(
                            EXP_SQ8, out=e[:, 0:glen * ICHUNK],
                            in0=mid[:, 0:glen * ICHUNK])
                        delay = O_DELAY_DVE
                        # the PE has no exp to shadow in this slot; keep it
                        # busy so the HAM clock gate stays open
                        pe_fill(12)
                    else:
                        nc.scalar.activation(e[:, 0:glen * ICHUNK],
                                             pm[:, 0:glen * ICHUNK],
                                             mybir.ActivationFunctionType.Exp,
                                             bias=expbias[:])
                        # chunk 0: the FIRST o-emission (po tile creation)
                        # must come after all staging tiles in psO1/psO2
                        # pool order; later o's drain at normal depth
                        delay = (max(5 - gidx, 1) if q == 0 else O_DELAY_ACT)
                        pe_fill(2)
                    for t in inject.pop(gidx, []):
                        t()
                    oqueue.append((gidx + delay,
                                   make_emit_o(q, e, gj0, glen), q))
                    flush_o(gidx)
                    gidx += 1
            flush_o(1 << 30)
            fin_back(*state["pend_back"])
    nc.compile()
    return nc


def _marshal(x_b, Wf, bf, Wg, bg, Wh, bh):
    """Build the per-core [65, 4176] input block."""
    xa = np.empty((C + 1, WN), dtype=np.float32)
    xa[0:C, 0:N] = x_b.reshape(C, N)
    xa[C, 0:N] = 1.0
    xa[0:C, N:N + C8] = Wf.T
    xa[C, N:N + C8] = bf
    xa[0:C, N + C8:N + 16] = Wg.T
    xa[C, N + C8:N + 16] = bg
    xa[0:C, N + 16:WN] = Wh.T
    xa[C, N + 16:WN] = bh
    return xa


LAST_RESULTS = None


def kernel(x, Wf, bf, Wg, bg, Wh, bh):
    global LAST_RESULTS
    x = np.asarray(x, dtype=np.float32)
    Wf = np.asarray(Wf, dtype=np.float32)
    bf = np.asarray(bf, dtype=np.float32)
    Wg = np.asarray(Wg, dtype=np.float32)
    bg = np.asarray(bg, dtype=np.float32)
    Wh = np.asarray(Wh, dtype=np.float32)
    bh = np.asarray(bh, dtype=np.float32)

    if "nc" not in _CACHE:
        _CACHE["nc"] = _build_nc()
    nc = _CACHE["nc"]

    in_maps = [{"inp": _marshal(x[b], Wf, bf, Wg, bg, Wh, bh)}
               for b in range(NCORES)]
    res = run_bass_kernel_spmd(nc, in_maps, list(range(NCORES)))
    LAST_RESULTS = res
    out = np.stack([res.results[b]["out"] for b in range(NCORES)], axis=0)
    return out.reshape(B, C, H, W).astype(np.float32)


# revision 36
# speedup vs baseline: 1.0424x; 1.0424x over previous
"""SAGAN-style attention block (nn_AttentionBlock) on 8 Trainium2 NeuronCores.

Problem (per sample): x [C=64, N=4096] (N = 64x64 spatial),
  f = Wf x + bf   [8, N]       g = Wg x + bg   [8, N]
  h = Wh x + bh   [64, N]
  s = f^T g       [N, N];  beta = softmax(s, axis=1)   (over j)
  o[c, i] = sum_j h[c, j] beta[i, j];   out = x + o
Sharding: pure data parallel over batch B=8 -> one sample per core.

Engine plan (all per core):
  - ACT is the natural bottleneck (16.7M exps at 153.6 G/s = 109us floor).
    ~25% of the exp work is offloaded to the DVE via two chained custom
    DVE ops registered at import time:
      EXP_BASE4_ANT: b = ((c0*s + c1)*s + c2)^4   (quadratic base, 6 stages)
      EXP_SQ8_ANT:   e = b^256                     (8 squarings)
    Together e = base^1024 ~ exp(s-12); rel err <1e-3 on significant
    entries (validated end-to-end: full-approx output rel err 2.5e-4).
  - All projections read x/weights as f32r via AP bitcast (no cast pass
    at all).  Input arrives as 8 x 512-col DMA pieces; f/g projections
    ([Wf;Wg] -> one [16,512] matmul per piece), hT tiles, and the main
    loop's first groups pipeline behind the DMA: first EXP at ~6us
    instead of ~34us.
  - A continuous dummy-matmul warmup stream from t~1us keeps the PE busy
    so the HAM clock gate (needs ~3.4us of sustained activity) opens
    before the real work, instead of 44us in.
  - PSUM (8 banks): psA 3 (s-macros + staged fg/hTa projections,
    single-buffered for chunk 0) + psB 3 (hTb staging j8-31, then joins
    s double-buffering from group 9) + po1 1 + po2 1 (o accumulators).
  - Finalize: merge accumulators + reciprocal on DVE; 1/den broadcast
    via DRAM bounce for chunks 0-6 (latency hidden), via a K=1 PE
    matmul into PSUM for the last chunk (kills the ~6.5us exposed tail);
    divide+residual on GpSimd (idle otherwise) for chunks 0-6.
  - o-matmuls of DVE-routed groups are deferred 3 groups (vs 1) in the
    emission pipeline so the PE never stalls on the slower DVE e-tiles.
    Accumulation order across j is irrelevant; j0/j1 (start) and j30/j31
    (stop) stay in the always-ACT groups 0 and 10.
"""
import sys

sys.path.insert(0, "/opt/trn_rl_repo")

import numpy as np
from contextlib import ExitStack

try:  # tracing hook is optional; provide a stub if absent
    import antenv.axon_hooks  # noqa: F401
except ImportError:
    import types
    import antenv
    _stub = types.ModuleType("antenv.axon_hooks")
    _stub._hook = None
    _stub.get_axon_ntff_profile_hook = lambda: _stub._hook
    def _set_hook(hook):
        _stub._hook = hook
    _stub.set_axon_ntff_profile_hook = _set_hook
    sys.modules["antenv.axon_hooks"] = _stub
    antenv.axon_hooks = _stub
    try:  # register the real ctypes NTFF hook (timing); degrade silently
        from trn_agent_boot.trn_boot import _ntff_profile_via_ctypes
        _stub._hook = _ntff_profile_via_ctypes("/opt/axon/libaxon_pjrt.so")
    except Exception:
        pass

import concourse.bass as bass  # noqa: F401  (bacc subclasses Bass)
import concourse.tile as tile
from concourse import bacc, mybir
from concourse.bass_utils import run_bass_kernel_spmd

# ---- custom DVE exp ops (registered at import) ----------------------------
from concourse.dve_ops import DveOp, OPS, CUSTOM_DVE_SPECS, _SUB_OPCODE_FOR_NAME
from concourse.dve_spec import Spec, Src0, C0, C1, C2, sq
from concourse.dve_spec import lower as _dve_lower
from concourse.dve_uop import DveOpSpec

F32 = mybir.dt.float32
F32R = mybir.dt.float32r
BF16 = mybir.dt.bfloat16
MMDT = BF16

EXPN = 1024.0  # exp(x) = base(x/EXPN)^EXPN, base = 1 + t + t^2/2
# base as a quadratic in s (x = s - 12 folded into the coefficients)
EB_C0 = float(0.5 / (EXPN * EXPN))
EB_C1 = float(1.0 / EXPN - 12.0 / (EXPN * EXPN))
EB_C2 = float(1.0 - 12.0 / EXPN + 72.0 / (EXPN * EXPN))


def _ref_exp_base4(in0, in1, s0, s1, imm2):
    x = in0.astype(np.float32)
    b = (np.float32(s0) * x + np.float32(s1)) * x + np.float32(imm2)
    return (b * b) * (b * b)


def _ref_sq8(in0, in1, s0, s1, imm2):
    b = in0.astype(np.float32)
    for _ in range(8):
        b = b * b
    return b


def _register_op(name, body, reference):
    if name in _SUB_OPCODE_FOR_NAME:  # idempotent across re-imports
        return next(op for op in OPS if op.name == name)
    spec = Spec(body=body, reference=reference)
    op = DveOp(name, spec, subdim=False, uops_sha={})
    object.__setattr__(op, "uops_sha", {
        ver: DveOpSpec(name=name, opcode=1, uops=_dve_lower(spec, ver=ver),
                       rd1_en=False).sha(ver)
        for ver in ("v3", "v4")})
    OPS.append(op)
    CUSTOM_DVE_SPECS[name] = spec
    _SUB_OPCODE_FOR_NAME[name] = max(_SUB_OPCODE_FOR_NAME.values()) + 1
    return op


EXP_BASE4 = _register_op(
    "EXP_BASE4_ANT", sq(sq((Src0 * C0 + C1) * Src0 + C2)), _ref_exp_base4)
EXP_SQ8 = _register_op(
    "EXP_SQ8_ANT", sq(sq(sq(sq(sq(sq(sq(sq(Src0)))))))), _ref_sq8)

B, C, H, W = 8, 64, 64, 64
N = H * W          # 4096
C8 = 8
NCORES = 8
ICHUNK = 512
NI = N // ICHUNK   # 8
JT = 128
NJ = N // JT       # 32
GROUP = 3
NPIECE = 8         # input x DMA pieces (512 cols each)
WN = N + 16 + C    # marshalled input width (x | wfg | wh)
NWARM = 7          # [128,512] dummy matmuls to open the HAM clock gate
# groups routed to the DVE exp (never 0 or 10: start/stop flags live there).
# chunks 0-1 stay on ACT: the DVE is busy with projection evac/replication
# during the prologue window.
DVE_GROUPS = {q: () for q in range(NI)}
O_DELAY_ACT = 1    # software-pipeline depth for o-matmul emission
O_DELAY_DVE = 3

_CACHE = {}


def _build_nc():
    nc = bacc.Bacc("TRN2", target_bir_lowering=False, debug=False,
                   num_devices=NCORES)
    inp = nc.dram_tensor("inp", [C + 1, WN], F32, kind="ExternalInput").ap()
    out = nc.dram_tensor("out", [C, N], F32, kind="ExternalOutput").ap()

    groups = []
    j0 = 0
    while j0 < NJ:
        groups.append((j0, min(GROUP, NJ - j0)))
        j0 += GROUP
    NG = len(groups)  # 11

    with tile.TileContext(nc) as tc:
        with ExitStack() as ctx:
            sb = ctx.enter_context(tc.tile_pool(name="sb", bufs=1))
            epool = ctx.enter_context(tc.tile_pool(name="ep", bufs=10))
            midp = ctx.enter_context(tc.tile_pool(name="mid", bufs=3))
            fin = ctx.enter_context(tc.tile_pool(name="fin", bufs=8))
            psA = ctx.enter_context(tc.tile_pool(name="psA", bufs=1, space="PSUM"))
            psB = ctx.enter_context(tc.tile_pool(name="psB", bufs=1, space="PSUM"))
            psO1 = ctx.enter_context(tc.tile_pool(name="psO1", bufs=1, space="PSUM"))
            psO2 = ctx.enter_context(tc.tile_pool(name="psO2", bufs=1, space="PSUM"))
            dram = ctx.enter_context(tc.tile_pool(name="dram", bufs=8, space="DRAM"))

            # ---- constants ----
            ones_f = sb.tile([128, 1], F32)
            nc.vector.memset(ones_f[:], 1.0)
            expbias = sb.tile([128, 1], F32)
            nc.vector.memset(expbias[:], -12.0)
            ones_c = sb.tile([1, C], F32)
            nc.vector.tensor_copy(ones_c[:], ones_f[0:1, 0:1].to_broadcast((1, C)))
            warm_src = sb.tile([128, 512], MMDT)
            nc.vector.tensor_copy(warm_src[:],
                                  ones_f[:, 0:1].to_broadcast((128, 512)))

            # ---- input DMA: x piece 0 first (it gates the first s-trio),
            # then weights, then the remaining pieces ----
            tin = sb.tile([C + 1, WN], F32)
            nc.sync.dma_start(tin[:, 0:ICHUNK], inp[:, 0:ICHUNK])
            nc.sync.dma_start(tin[:, N:WN], inp[:, N:WN])
            for c in range(1, NPIECE):
                cs = slice(c * ICHUNK, (c + 1) * ICHUNK)
                nc.sync.dma_start(tin[:, cs], inp[:, cs])
            # bf16 copies for the projection matmuls (f32r would need a
            # rounding producer: the BIR verifier rejects plain bitcasts).
            # Measured output rel-L2 with all-bf16 projections is ~3e-3,
            # well inside the 2e-2 gate.
            # [Wf | pad | Wg] stationary: f lands at PSUM partitions 0-7,
            # g at 32-39 (PSUM reads must start at a 32-aligned partition)
            wfg_b = sb.tile([C + 1, 40], MMDT)
            nc.vector.memset(wfg_b[:], 0.0)
            nc.vector.tensor_copy(wfg_b[:, 0:C8], tin[:, N:N + C8])
            nc.vector.tensor_copy(wfg_b[:, 32:40], tin[:, N + C8:N + 16])
            wh_b = sb.tile([C + 1, C], MMDT)
            nc.vector.tensor_copy(wh_b[:], tin[:, N + 16:WN])
            xb = sb.tile([C + 1, N], MMDT)

            # ---- SBUF landing zones ----
            # f/g replicated to partition offsets 0/32/64 (quadrant packing
            # needs both s-matmul operands at base partition 0/32/64)
            f_sb = sb.tile([72, N], MMDT)
            g_sb = sb.tile([72, N], MMDT)
            hT = sb.tile([JT, NJ, C + 1], MMDT)
            res = sb.tile([C, N], F32)

            # ---- warmup: continuous PE activity to open the HAM gate ----
            warm_ps = psO2.tile([JT, 512], F32, tag="po2", name="warm")
            for _ in range(NWARM):
                nc.tensor.matmul(warm_ps[:], warm_src[:, 0:JT], warm_src[:],
                                 start=True, stop=True)

            def pe_fill(n):
                # dummy stationary loads: pure PE activity (no PSUM/output)
                # to keep the HAM activity monitor from closing the clock
                # gate during engine-wait windows
                for _ in range(n):
                    nc.tensor.ldweights(warm_src[:, 0:JT])

            # ---- staged projections (psA pool, serialized with s-macros) ----
            def fg_piece(c):
                cs = slice(c * ICHUNK, (c + 1) * ICHUNK)
                nc.vector.tensor_copy(xb[:, cs], tin[:, cs])
                pool, tg = (psO1, "po1") if c % 2 == 0 else (psO2, "po2")
                pp = pool.tile([40, ICHUNK], F32, tag=tg, name=f"fg{c}")
                nc.tensor.matmul(pp[:], wfg_b[:], xb[:, cs],
                                 start=True, stop=True)
                # evac (the fp32->bf16 cast point); split ACT/DVE for the
                # first two pieces, which gate the first s-trio
                if c < 2:
                    nc.scalar.copy(f_sb[0:C8, cs], pp[0:C8, :])
                else:
                    nc.vector.tensor_copy(f_sb[0:C8, cs], pp[0:C8, :])
                nc.vector.tensor_copy(g_sb[0:C8, cs], pp[32:40, :])
                for r in (32, 64):
                    nc.vector.tensor_copy(f_sb[r:r + C8, cs], f_sb[0:C8, cs])
                    nc.vector.tensor_copy(g_sb[r:r + C8, cs], g_sb[0:C8, cs])

            # hT staging: four [128, 8*64] quarter-tiles through psB (its
            # s-macro duty starts only at group 6), serialized by the pool
            hq = {}

            def hT_mm(t, half):
                if half == 0:
                    hq[t] = psB.tile([JT, 8 * C], F32, tag="m", name=f"hq{t}")
                for j in range(8 * t + 4 * half, 8 * t + 4 * half + 4):
                    nc.tensor.matmul(hq[t][:, (j - 8 * t) * C:(j - 8 * t + 1) * C],
                                     xb[:, j * JT:(j + 1) * JT],
                                     wh_b[:], start=True, stop=True)

            def hT_evac(t):
                nc.vector.tensor_copy(
                    hT[:, 8 * t:8 * t + 8, 0:C],
                    hq[t][:].rearrange("p (a b) -> p a b", a=8))

            fg_piece(0)
            fg_piece(1)
            fg_piece(2)
            hT_mm(0, 0)
            hT_mm(0, 1)
            hT_evac(0)
            nc.vector.tensor_copy(hT[:, :, C:C + 1],
                                  ones_f[:].to_broadcast((JT, NJ, 1)))

            # prologue tasks injected between chunk-0 group emissions,
            # keyed by global group index
            inject = {
                0: [lambda: fg_piece(3), lambda: hT_mm(1, 0)],
                1: [lambda: fg_piece(4), lambda: hT_mm(1, 1),
                    lambda: hT_evac(1)],
                2: [lambda: fg_piece(5), lambda: hT_mm(2, 0)],
                3: [lambda: fg_piece(6), lambda: hT_mm(2, 1),
                    lambda: hT_evac(2)],
                4: [lambda: fg_piece(7), lambda: hT_mm(3, 0)],
                5: [lambda: hT_mm(3, 1), lambda: hT_evac(3)],
            }

            # ---- finalize helpers ----
            def fin_front(po, q):
                # merge the two o-accumulator halves (an engine may read
                # only ONE non-scalar PSUM input per instruction)
                oc = fin.tile([C + 1, ICHUNK], F32, tag="oc", name=f"oc{q}")
                nc.vector.tensor_copy(oc[:], po[0][:])
                nc.vector.tensor_add(oc[:], oc[:], po[1][:])
                # reciprocal of the WHOLE oc tile (base partition 0 keeps
                # the custom op well-behaved); only row C (the denominator)
                # is ever read -- rows 0-63 are 1/o garbage, never used
                ri = fin.tile([C + 1, ICHUNK], F32, tag="ri", name=f"ri{q}")
                nc.vector.reciprocal_approx_fast(ri[:], oc[:])
                r = ri[C:C + 1, :]
                if q < NI - 1:
                    scr = dram.tile([1, ICHUNK], F32, tag="scr", name=f"scr{q}")
                    nc.sync.dma_start(scr[:], r)
                    rb = fin.tile([C, ICHUNK], F32, tag="rb", name=f"rb{q}")
                    nc.sync.dma_start(rb[:], scr[:].to_broadcast((C, ICHUNK)))
                else:
                    # tail chunk: broadcast 1/den via a K=1 fp32 PE matmul
                    # into PSUM (psA is free once the last s-macro is read;
                    # fp32 streams 4 cyc/col but the PE is idle here).
                    # matmul operands must share a base partition -> move
                    # the reciprocal row to partition 0 first.
                    r0 = fin.tile([1, ICHUNK], F32, tag="r0", name="r0")
                    nc.vector.tensor_copy(r0[:], r)
                    rb = psA.tile([C, ICHUNK], F32, tag="m", name="rbps")
                    nc.tensor.matmul(rb[:], ones_c[:], r0[:],
                                     start=True, stop=True)
                return (oc, rb, q)

            def fin_back(oc, rb, q):
                qs = slice(q * ICHUNK, (q + 1) * ICHUNK)
                eng = nc.gpsimd if q < NI - 1 else nc.vector
                eng.tensor_mul(res[:, qs], oc[0:C, :], rb[:])
                eng.tensor_add(res[:, qs], res[:, qs], tin[0:C, qs])
                nc.sync.dma_start(out[:, qs], res[:, qs])

            # ---- main loop with deferred-o scheduler ----
            # po tiles are allocated lazily at the FIRST o-matmul emission
            # of each chunk: a pool's buffer rotation follows tile-creation
            # order, so allocating them at the chunk-loop top would place
            # them BEFORE later-injected staging tiles in psO1/psO2 and
            # deadlock (staging would wait on the chunk's merge).
            po_of = {}
            done_o = {q: 0 for q in range(NI)}
            oqueue = []  # (due_gidx, emit_fn, q) in queue order
            state = {"pend_back": None}

            def get_po(q):
                if q not in po_of:
                    po_of[q] = [
                        psO1.tile([C + 1, ICHUNK], F32, tag="po1",
                                  name=f"po1_{q}"),
                        psO2.tile([C + 1, ICHUNK], F32, tag="po2",
                                  name=f"po2_{q}"),
                    ]
                return po_of[q]

            def make_emit_o(q, e, j0_, glen_):
                def emit():
                    po = get_po(q)
                    for k in range(glen_):
                        j = j0_ + k
                        nc.tensor.matmul(po[j % 2][:], hT[:, j, :],
                                         e[:, k * ICHUNK:(k + 1) * ICHUNK],
                                         start=(j < 2), stop=(j >= NJ - 2))
                return emit

            def flush_o(now):
                while oqueue and oqueue[0][0] <= now:
                    _, fn, oq = oqueue.pop(0)
                    fn()
                    done_o[oq] += 1
                    if done_o[oq] == NG:
                        front = fin_front(po_of[oq], oq)
                        if state["pend_back"] is not None:
                            fin_back(*state["pend_back"])
                        state["pend_back"] = front

            gidx = 0
            for q in range(NI):
                qs = slice(q * ICHUNK, (q + 1) * ICHUNK)
                dset = DVE_GROUPS.get(q, ())
                for gi, (gj0, glen) in enumerate(groups):
                    # chunk 0 groups 0-5: psA single-buffered (psB stages hT)
                    if gidx < 6:
                        pool = psA
                    else:
                        pool = psB if (gidx - 6) % 2 == 0 else psA
                    pm = pool.tile([JT, GROUP * ICHUNK], F32, tag="m")
                    for k in range(glen):
                        j = gj0 + k
                        nc.tensor.matmul(
                            pm[:, k * ICHUNK:(k + 1) * ICHUNK],
                            g_sb[32 * k:32 * k + C8, j * JT:(j + 1) * JT],
                            f_sb[32 * k:32 * k + C8, qs],
                            start=True, stop=True)
                    e = epool.tile([JT, GROUP * ICHUNK], MMDT, tag="e")
                    if gi in dset:
                        mid = midp.tile([JT, GROUP * ICHUNK], F32, tag="mid")
                        nc.vector._custom_dve(
                            EXP_BASE4, out=mid[:, 0:glen * ICHUNK],
                            in0=pm[:, 0:glen * ICHUNK],
                            s0=EB_C0, s1=EB_C1, imm2=EB_C2)
                        nc.vector._custom_dve(
                            EXP_SQ8, out=e[:, 0:glen * ICHUNK],
                            in0=mid[:, 0:glen * ICHUNK])
                        delay = O_DELAY_DVE
                    else:
                        nc.scalar.activation(e[:, 0:glen * ICHUNK],
                                             pm[:, 0:glen * ICHUNK],
                                             mybir.ActivationFunctionType.Exp,
                                             bias=expbias[:])
                        # chunk 0: the FIRST o-emission (po tile creation)
                        # must come after all staging tiles in psO1/psO2
                        # pool order; later o's drain at normal depth
                        delay = (max(5 - gidx, 1) if q == 0 else O_DELAY_ACT)
                    for t in inject.pop(gidx, []):
                        t()
                    oqueue.append((gidx + delay,
                                   make_emit_o(q, e, gj0, glen), q))
                    flush_o(gidx)
                    gidx += 1
            flush_o(1 << 30)
            fin_back(*state["pend_back"])
    nc.compile()
    return nc


def _marshal(x_b, Wf, bf, Wg, bg, Wh, bh):
    """Build the per-core [65, 4176] input block."""
    xa = np.empty((C + 1, WN), dtype=np.float32)
    xa[0:C, 0:N] = x_b.reshape(C, N)
    xa[C, 0:N] = 1.0
    xa[0:C, N:N + C8] = Wf.T
    xa[C, N:N + C8] = bf
    xa[0:C, N + C8:N + 16] = Wg.T
    xa[C, N + C8:N + 16] = bg
    xa[0:C, N + 16:WN] = Wh.T
    xa[C, N + 16:WN] = bh
    return xa


LAST_RESULTS = None


def kernel(x, Wf, bf, Wg, bg, Wh, bh):
    global LAST_RESULTS
    x = np.asarray(x, dtype=np.float32)
    Wf = np.asarray(Wf, dtype=np.float32)
    bf = np.asarray(bf, dtype=np.float32)
    Wg = np.asarray(Wg, dtype=np.float32)
    bg = np.asarray(bg, dtype=np.float32)
    Wh = np.asarray(Wh, dtype=np.float32)
    bh = np.asarray(bh, dtype=np.float32)

    if "nc" not in _CACHE:
        _CACHE["nc"] = _build_nc()
    nc = _CACHE["nc"]

    in_maps = [{"inp": _marshal(x[b], Wf, bf, Wg, bg, Wh, bh)}
               for b in range(NCORES)]
    res = run_bass_kernel_spmd(nc, in_maps, list(range(NCORES)))
    LAST_RESULTS = res
    out = np.stack([res.results[b]["out"] for b in range(NCORES)], axis=0)
    return out.reshape(B, C, H, W).astype(np.float32)
